# revision 20
# baseline (speedup 1.0000x reference)
"""Multi-head self-attention Trainium2 kernel (8-core data-parallel over batch).

Layout strategy (per core = one batch element):
  - host pre-transposes x -> xT [H, S] and weights -> W^T [in, out], so every
    matmul contracts over the SBUF partition axis with zero on-chip transposes.
  - qT, kT computed as [out, seq]; v computed natural [seq, out] with a ones
    column appended per head (v_aug) so the ctx matmul also produces softmax
    denominators for free (M=65 output rows; row 64 = sum_k probs).
  - scores computed transposed [k, q]; mask+scale+exp fused into a single
    ScalarE activation per tile: exp(smt[k] * s + cmt[k]) where smt=mask*scale,
    cmt=(1-mask)*(-10000). Two heads per PSUM tile via PE row-tiling (K=64).
  - softmax denominators reciprocated via Ln+Exp at the tail, broadcast across
    64 partitions with a tiny indicator matmul, applied as one elementwise mult.
  - out projection contracts ctxT directly; bo added on host after gather.
"""

import sys

for _p in ("/opt/trn_rl_repo", "/root/.axon_site/_ro/trn_rl_repo"):
    if _p not in sys.path:
        sys.path.append(_p)

import numpy as np

import concourse.bass as bass
import concourse.tile as tile
from concourse import mybir
from concourse.bass_utils import run_bass_kernel_spmd

F32 = mybir.dt.float32
F32R = mybir.dt.float32r
EXP = mybir.ActivationFunctionType.Exp
LN = mybir.ActivationFunctionType.Ln
COPY = mybir.ActivationFunctionType.Copy

B, S, H = 8, 1024, 768
NH, HD = 12, 64
NIC = H // 128          # 6 contraction chunks of 128
NSB = S // 128          # 8 seq blocks of 128
NPAIR = NH // 2         # 6 head pairs
VW = NH * (HD + 1)      # 780: v_aug columns per k-chunk (65 per head)
SCALE = HD ** -0.5

# matmul datapath dtype: float32r streams 1 row/cycle (vs 4 for float32) on the
# PE at reduced multiply precision; the BIR verifier requires every producer of
# a matmul operand to emit the same dtype, so it is threaded through all tiles.
import os as _os
_MMDT_NAME = _os.environ.get("MM_DTYPE", "f32r")
MMDT = {"f32": F32, "f32r": F32R, "bf16": mybir.dt.bfloat16}[_MMDT_NAME]


def _split_excess_waits(nc, max_waits=1):
    """The pinned walrus rejects >1 semaphore wait per instruction
    ("Too many sync wait commands"). Waits are pre-conditions, so move the
    excess onto NOPs inserted immediately before the instruction."""
    for f in nc.m.functions:
        for bb in f.blocks:
            new_insts = []
            for inst in bb.instructions:
                w = inst.sync_info.on_wait if inst.sync_info else None
                if w and len(w) > max_waits:
                    chunks = [w[i:i + max_waits] for i in range(0, len(w), max_waits)]
                    for ci, chunk in enumerate(chunks[:-1]):
                        new_insts.append(mybir.InstNoOp(
                            name=f"{inst.name}_waitsplit_{ci}",
                            engine=inst.engine,
                            sync_info=mybir.SyncInfo(on_wait=list(chunk), on_update=[]),
                            bass_nofuse=True,
                        ))
                    inst.sync_info.on_wait = list(chunks[-1])
                new_insts.append(inst)
            bb.instructions[:] = new_insts


def _emit(ctx, tc, nc, d, with_bq, with_bk, with_bv):
    ts = bass.ts

    p_w = ctx.enter_context(tc.tile_pool(name="w", bufs=2))
    p_x = ctx.enter_context(tc.tile_pool(name="x", bufs=1))
    p_qkv = ctx.enter_context(tc.tile_pool(name="qkv", bufs=1))
    p_small = ctx.enter_context(tc.tile_pool(name="small", bufs=1))
    p_probs = ctx.enter_context(tc.tile_pool(name="probs", bufs=2))
    p_ctx = ctx.enter_context(tc.tile_pool(name="ctx", bufs=1))
    p_out = ctx.enter_context(tc.tile_pool(name="out", bufs=2))
    ps_mm = ctx.enter_context(tc.tile_pool(name="psmm", bufs=2, space="PSUM"))
    ps_s = ctx.enter_context(tc.tile_pool(name="pssc", bufs=1, space="PSUM"))

    # ---- input DMAs ----
    xT_t = p_x.tile([128, NIC * S], MMDT, tag="x")
    for ic in range(NIC):
        nc.sync.dma_start(out=xT_t[:, ts(ic, S)], in_=d["xT"][ts(ic, 128), :])

    w_tiles = {}

    def load_w(wname):
        wt = p_w.tile([128, NIC * H], MMDT, tag="w", name=wname)
        w_tiles[wname] = wt
        for ic in range(NIC):
            nc.sync.dma_start(out=wt[:, ts(ic, H)], in_=d[wname][ts(ic, 128), :])

    load_w("wqT")
    load_w("wkT")

    smt_t = p_small.tile([128, NSB], F32, tag="smt")
    nc.sync.dma_start(out=smt_t, in_=d["smt"][:, :])
    cmt_t = p_small.tile([128, NSB], F32, tag="cmt")
    nc.sync.dma_start(out=cmt_t, in_=d["cmt"][:, :])
    ind_t = p_small.tile([NH, H], MMDT, tag="ind")
    nc.sync.dma_start(out=ind_t, in_=d["ind"][:, :])
    bias_tiles = {}
    for bname, flag in (("bqT", with_bq), ("bkT", with_bk), ("bvT", with_bv)):
        if flag:
            bt = p_small.tile([128, NIC], F32, tag=bname)
            nc.sync.dma_start(out=bt, in_=d[bname][:, :])
            bias_tiles[bname] = bt

    sums_t = p_small.tile([NH, S], MMDT, tag="sums")

    # ---- q/k projections -> qT/kT [128, ob*S + s] (head h at chunk h//2, partitions (h%2)*64) ----
    qT_t = p_qkv.tile([128, NIC * S], MMDT, tag="qT")
    kT_t = p_qkv.tile([128, NIC * S], MMDT, tag="kT")
    for wname, dst, bname, flag in (
        ("wqT", qT_t, "bqT", with_bq), ("wkT", kT_t, "bkT", with_bk),
    ):
        wt = w_tiles[wname]
        for ob in range(NIC):
            for sh in range(2):
                ps = ps_mm.tile([128, 512], F32, tag="mm")
                for ic in range(NIC):
                    nc.tensor.matmul(
                        ps,
                        wt[:, ic * H + ob * 128: ic * H + (ob + 1) * 128],
                        xT_t[:, ic * S + sh * 512: ic * S + (sh + 1) * 512],
                        start=(ic == 0), stop=(ic == NIC - 1),
                    )
                dst_ap = dst[:, ob * S + sh * 512: ob * S + (sh + 1) * 512]
                if flag:
                    nc.vector.tensor_scalar_add(
                        out=dst_ap, in0=ps, scalar1=bias_tiles[bname][:, ob:ob + 1])
                else:
                    nc.vector.tensor_copy(out=dst_ap, in_=ps)

    # ---- v projection -> v_aug [128, kc*VW + h*65 + d], column 64 of each head = 1.0 ----
    load_w("wvT")
    v_t = p_qkv.tile([128, NSB * VW], MMDT, tag="v")
    v_view = v_t.rearrange("p (kc h e) -> p kc h e", kc=NSB, h=NH)
    nc.sync.dma_start(
        out=v_view[:, :, :, HD:HD + 1],
        in_=d["vones"][:, :].rearrange("p (kc h e) -> p kc h e", kc=NSB, h=NH))
    wv = w_tiles["wvT"]
    for sb in range(NSB):
        for oh in range(2):
            o0, ow = (0, 512) if oh == 0 else (512, 256)
            ps = ps_mm.tile([128, 512], F32, tag="mm")
            for ic in range(NIC):
                nc.tensor.matmul(
                    ps[:, 0:ow],
                    xT_t[:, ic * S + sb * 128: ic * S + (sb + 1) * 128],
                    wv[:, ic * H + o0: ic * H + o0 + ow],
                    start=(ic == 0), stop=(ic == NIC - 1),
                )
            h0, hn = (0, 8) if oh == 0 else (8, 4)
            src = ps[:, 0:ow].rearrange("p (h e) -> p h e", e=HD)
            dst_ap = v_view[:, sb, h0:h0 + hn, 0:HD]
            if with_bv:
                # bv folded later via ctx path is wrong pre-softmax; add here.
                bv_b = d["bv_bc"]
                nc.vector.tensor_add(out=dst_ap, in0=src, in1=bv_b[:, sb, h0:h0 + hn, :])
            else:
                nc.vector.tensor_copy(out=dst_ap, in_=src)

    # ---- attention: per head pair ----
    load_w("woT")  # overlaps with attention compute
    # per-head softmax denominators: compute engines can only address start
    # partitions in {0,32,64,96}, so stage even heads on partition 0 and odd
    # heads on partition 64 (one column block per pair), then two SBUF->SBUF
    # DMAs restack to [NH, S]: row j<6 = head 2j, row j>=6 = head 2(j-6)+1.
    # The host-built indicator matrix uses the same permutation.
    # Reuses xT's SBUF slot (tag "x"): xT is dead once the v projection is done.
    stage_t = p_x.tile([128, NPAIR * S], MMDT, tag="x")
    ctx_t = p_ctx.tile([128, NIC * S], MMDT, tag="ctxT")
    for pr in range(NPAIR):
        ob = pr  # qT/kT chunk holding heads 2pr (parts 0:64) and 2pr+1 (parts 64:128)
        ctxA = ps_mm.tile([HD + 1, S], F32, tag="mm")
        ctxB = ps_mm.tile([HD + 1, S], F32, tag="mm")
        for kc in range(NSB):
            sc = ps_s.tile([128, 2048], F32, tag="scores")
            for qh in range(2):
                nc.tensor.matmul(
                    sc[:, qh * 512:(qh + 1) * 512],
                    kT_t[0:64, ob * S + kc * 128: ob * S + (kc + 1) * 128],
                    qT_t[0:64, ob * S + qh * 512: ob * S + (qh + 1) * 512],
                    start=True, stop=True, tile_position=(0, 0),
                )
                nc.tensor.matmul(
                    sc[:, 1024 + qh * 512: 1024 + (qh + 1) * 512],
                    kT_t[64:128, ob * S + kc * 128: ob * S + (kc + 1) * 128],
                    qT_t[64:128, ob * S + qh * 512: ob * S + (qh + 1) * 512],
                    start=True, stop=True, tile_position=(64, 0),
                )
            probs = p_probs.tile([128, 2048], MMDT, tag="probs")
            nc.scalar.activation(
                out=probs, in_=sc, func=EXP,
                scale=smt_t[:, kc:kc + 1], bias=cmt_t[:, kc:kc + 1],
            )
            for hh, ctx_ps in ((0, ctxA), (1, ctxB)):
                voff = kc * VW + (2 * pr + hh) * (HD + 1)
                for qh in range(2):
                    nc.tensor.matmul(
                        ctx_ps[:, qh * 512:(qh + 1) * 512],
                        v_t[:, voff: voff + HD + 1],
                        probs[:, hh * 1024 + qh * 512: hh * 1024 + (qh + 1) * 512],
                        start=(kc == 0), stop=(kc == NSB - 1),
                    )
        for hh, ctx_ps in ((0, ctxA), (1, ctxB)):
            h = 2 * pr + hh
            nc.vector.tensor_copy(
                out=ctx_t[hh * 64:(hh + 1) * 64, ob * S:(ob + 1) * S],
                in_=ctx_ps[0:HD, :],
            )
            nc.scalar.activation(
                out=stage_t[hh * 64:hh * 64 + 1, pr * S:(pr + 1) * S],
                in_=ctx_ps[HD:HD + 1, :], func=COPY)

    # ---- reciprocal of denominators: 1/s = exp(-ln(s)) ----
    for hh in range(2):
        nc.sync.dma_start(
            out=sums_t[hh * NPAIR:(hh + 1) * NPAIR, :],
            in_=stage_t[hh * 64:hh * 64 + 1, :])
    nc.scalar.activation(out=sums_t, in_=sums_t, func=LN)
    nc.scalar.activation(out=sums_t, in_=sums_t, func=EXP, scale=-1.0)

    # ---- normalize ctxT: broadcast recip across the 64 d-rows of each head ----
    for ic in range(NIC):
        bc = ps_mm.tile([128, S], F32, tag="mm")
        for qh in range(2):
            nc.tensor.matmul(
                bc[:, qh * 512:(qh + 1) * 512],
                ind_t[:, ts(ic, 128)],
                sums_t[:, qh * 512:(qh + 1) * 512],
                start=True, stop=True,
            )
        nc.vector.tensor_mul(
            out=ctx_t[:, ts(ic, S)], in0=ctx_t[:, ts(ic, S)], in1=bc)

    # ---- out projection ----
    wo = w_tiles["woT"]
    for sb in range(NSB):
        ps = ps_mm.tile([128, H], F32, tag="mm")
        for ic in range(NIC):
            lhs = ctx_t[:, ic * S + sb * 128: ic * S + (sb + 1) * 128]
            nc.tensor.matmul(ps[:, 0:512], lhs, wo[:, ic * H: ic * H + 512],
                             start=(ic == 0), stop=(ic == NIC - 1))
            nc.tensor.matmul(ps[:, 512:H], lhs, wo[:, ic * H + 512: (ic + 1) * H],
                             start=(ic == 0), stop=(ic == NIC - 1))
        ot = p_out.tile([128, H], F32, tag="out")
        nc.vector.tensor_copy(out=ot, in_=ps)
        nc.sync.dma_start(out=d["out"][ts(sb, 128), :], in_=ot)


def declare_params(nc, with_bq=False, with_bk=False, with_bv=False):
    d = {
        "xT": nc.declare_dram_parameter("xT", [H, S], MMDT, isOutput=False).ap(),
        "wqT": nc.declare_dram_parameter("wqT", [H, H], MMDT, isOutput=False).ap(),
        "wkT": nc.declare_dram_parameter("wkT", [H, H], MMDT, isOutput=False).ap(),
        "wvT": nc.declare_dram_parameter("wvT", [H, H], MMDT, isOutput=False).ap(),
        "woT": nc.declare_dram_parameter("woT", [H, H], MMDT, isOutput=False).ap(),
        "smt": nc.declare_dram_parameter("smt", [128, NSB], F32, isOutput=False).ap(),
        "cmt": nc.declare_dram_parameter("cmt", [128, NSB], F32, isOutput=False).ap(),
        "ind": nc.declare_dram_parameter("ind", [NH, H], MMDT, isOutput=False).ap(),
        "vones": nc.declare_dram_parameter("vones", [128, NSB * NH], MMDT, isOutput=False).ap(),
        "out": nc.declare_dram_parameter("out", [S, H], F32, isOutput=True).ap(),
    }
    if with_bq:
        d["bqT"] = nc.declare_dram_parameter("bqT", [128, NIC], F32, isOutput=False).ap()
    if with_bk:
        d["bkT"] = nc.declare_dram_parameter("bkT", [128, NIC], F32, isOutput=False).ap()
    if with_bv:
        bvb = nc.declare_dram_parameter("bv_bc", [128, NSB * NH * HD], F32, isOutput=False)
        d["bv_bc"] = bvb.ap().rearrange("p (kc h e) -> p kc h e", kc=NSB, h=NH)
    return d


def build_nc(with_bq=False, with_bk=False, with_bv=False, split_waits=True):
    nc = bass.Bass("TRN2", target_bir_lowering=False, debug=False)
    d = declare_params(nc, with_bq, with_bk, with_bv)

    from contextlib import ExitStack
    with tile.TileContext(nc) as tc, ExitStack() as es:
        _emit(es, tc, nc, d, with_bq, with_bk, with_bv)
    if split_waits:
        _split_excess_waits(nc)
    return nc


def make_in_maps(x, attention_mask, Wq, bq, Wk, bk, Wv, bv, Wo, bo):
    with_bq = bool(np.any(bq)) if bq is not None else False
    with_bk = bool(np.any(bk)) if bk is not None else False
    with_bv = bool(np.any(bv)) if bv is not None else False

    if _MMDT_NAME == "bf16":
        import ml_dtypes
        mm_np = ml_dtypes.bfloat16
    else:
        mm_np = np.float32
    shared = {
        "wqT": np.ascontiguousarray(np.asarray(Wq, np.float32).T).astype(mm_np),
        "wkT": np.ascontiguousarray(np.asarray(Wk, np.float32).T).astype(mm_np),
        "wvT": np.ascontiguousarray(np.asarray(Wv, np.float32).T).astype(mm_np),
        "woT": np.ascontiguousarray(np.asarray(Wo, np.float32).T).astype(mm_np),
    }
    # row j of sums_t holds head 2j (j<6) or 2(j-6)+1 (j>=6); ind selects it
    ind = np.zeros((NH, H), np.float32)
    for j in range(NH):
        h = 2 * j if j < NPAIR else 2 * (j - NPAIR) + 1
        ind[j, h * HD:(h + 1) * HD] = 1.0
    shared["ind"] = ind.astype(mm_np)
    shared["vones"] = np.ones((128, NSB * NH), np.float32).astype(mm_np)
    if with_bq:
        shared["bqT"] = np.ascontiguousarray(
            np.asarray(bq, np.float32).reshape(NIC, 128).T)
    if with_bk:
        shared["bkT"] = np.ascontiguousarray(
            np.asarray(bk, np.float32).reshape(NIC, 128).T)
    if with_bv:
        # [128, kc*NH*HD] replicated bias in the v_aug head layout (minus ones col)
        bvv = np.asarray(bv, np.float32).reshape(NH, HD)
        bvb = np.broadcast_to(bvv[None, None], (128, NSB, NH, HD))
        shared["bv_bc"] = np.ascontiguousarray(bvb.reshape(128, NSB * NH * HD))

    x = np.asarray(x, np.float32)
    mask = np.asarray(attention_mask)
    in_maps = []
    for c in range(B):
        m = mask[c].astype(np.float32)
        smt = np.ascontiguousarray((m * SCALE).reshape(NSB, 128).T)
        cmt = np.ascontiguousarray(((1.0 - m) * -10000.0).reshape(NSB, 128).T)
        in_maps.append({
            "xT": np.ascontiguousarray(x[c].T).astype(mm_np),
            "smt": smt, "cmt": cmt, **shared,
        })
    return in_maps, (with_bq, with_bk, with_bv)


_nc_cache = {}


def kernel(x, attention_mask, Wq, bq, Wk, bk, Wv, bv, Wo, bo):
    in_maps, flags = make_in_maps(x, attention_mask, Wq, bq, Wk, bk, Wv, bv, Wo, bo)
    if flags not in _nc_cache:
        _nc_cache[flags] = build_nc(*flags)
    nc = _nc_cache[flags]
    res = run_bass_kernel_spmd(nc, in_maps, core_ids=list(range(B)))
    out = np.stack([res.results[c]["out"] for c in range(B)], axis=0)
    out = out + np.asarray(bo, np.float32)[None, None, :]
    return out.astype(np.float32)
